# revision 43
# baseline (speedup 1.0000x reference)
"""Trainium2 Bass kernel for the cross-attention block.

Sharding: B=2 batches x 4 head-pairs = 8 cores. Core i handles batch i//4 and
heads (2*(i%4), 2*(i%4)+1). Each core computes its heads' full NxM attention
and a partial merge projection; the host sums the 4 partials per batch.

Host-side preprocessing (cheap numpy):
  - masked source rows are filtered out entirely (softmax over m is
    permutation-invariant; masked rows contribute nothing), padded to a
    multiple of 128 with zero rows + a 0 indicator so pads drop out of both
    numerator and denominator.
  - x / source / weights are pre-transposed AND cast to bf16 so the device
    never transposes and DMA bytes are halved.

Device pipeline per core (all matmuls bf16 in / fp32 psum out):
  QT[hd=128, n]  = wq.T @ xT          (4 c-chunks accumulated)
  KT[hd=128, m]  = wk.T @ sT
  V_aug[m, 130]  = sT.T @ wv | ind    (cols 64/129 hold the valid-row mask)
  per (n-tile 512, head, group of 2 m-chunks):
      S[m=128, n=512] = KT_h-chunk.T @ QT_h     (K = 64 head dims)
      e = Exp(0.125 * S)   -> bf16 SBUF         (ScalarE, the bottleneck)
  per (head, n-chunk 128): A[n, 65] = sum_mc e[mc].T @ V_aug_h[mc]
      col 64 = softmax denominator (per-partition scalar!)
      r = 1/A[:,64] (DVE reciprocal), aggT[n, d] = A[:,0:64] * r  (DVE)
  transpose aggT[n, 128] -> agg[d=128, n] per n-chunk (PE, identity matmul)
  out[n, 512] = agg.T @ wm  -> SBUF copy (Pool) -> DMA per n-chunk

Engine budget (TimelineSim cost model): ACT = 32 exp x ~1.04us = 33.2us is
the floor; PE ~= 32us hides under it; DVE ~14us; Pool ~13us; DMA ~20us.
"""

import math
import sys

import numpy as np

try:
    import concourse.bass as bass
except ImportError:  # pragma: no cover - fresh grading dir safety
    sys.path.insert(0, "/opt/trn_rl_repo")
    import concourse.bass as bass

import concourse.mybir as mybir
from concourse import bacc
import concourse.tile as tile
from concourse.bass_utils import run_bass_kernel_spmd
from concourse.masks import make_identity

F32 = mybir.dt.float32
BF16 = mybir.dt.bfloat16
NP_BF16 = mybir.dt.np(mybir.dt.bfloat16)
EXP = mybir.ActivationFunctionType.Exp

DIM = 512
N_SEQ = 2048
NT_W = 512          # n-tile width for the score/exp stream
P = 128


def build_bass(m_pad: int):
    """Build the per-core Bass program. m_pad: padded valid-source length."""
    m_ch = m_pad // P            # m chunks of 128
    n_groups = (m_ch + 1) // 2   # exp groups of 2 m-chunks
    n_tiles = N_SEQ // NT_W      # 4
    c_ch = DIM // P              # 4 contraction chunks for projections
    NC = NT_W // P               # n-chunks of 128 per n-tile

    nc = bacc.Bacc("TRN2", target_bir_lowering=False, debug=False,
                   num_devices=8)

    xT = nc.dram_tensor("xT", [P, c_ch, N_SEQ], BF16, kind="ExternalInput")
    sT = nc.dram_tensor("sT", [P, c_ch, m_pad], BF16, kind="ExternalInput")
    wq = nc.dram_tensor("wq", [P, c_ch, P], BF16, kind="ExternalInput")
    wk = nc.dram_tensor("wk", [P, c_ch, P], BF16, kind="ExternalInput")
    # wvmi: wv [P, 512] | wm [P, 512] | ind [P, m_ch]
    wvmi = nc.dram_tensor("wvmi", [P, DIM + DIM + m_ch], BF16,
                          kind="ExternalInput")
    # bf16 output halves the out-DMA; the host sums the 4 partials in fp32
    out = nc.dram_tensor("out", [N_SEQ, DIM], BF16, kind="ExternalOutput")

    with tile.TileContext(nc) as tc:
        with (
            tc.tile_pool(name="consts", bufs=1) as consts,
            tc.tile_pool(name="bigin", bufs=1) as bigin,
            tc.tile_pool(name="proj", bufs=1) as proj,
            tc.tile_pool(name="exps", bufs=3) as exps,
            tc.tile_pool(name="aggt", bufs=2) as aggt,
            tc.tile_pool(name="aggsb", bufs=2) as aggsb,
            tc.tile_pool(name="outp", bufs=2) as outp,
            tc.tile_pool(name="small", bufs=4) as small,
            tc.tile_pool(name="sc_ps", bufs=2, space="PSUM") as sc_ps,
            tc.tile_pool(name="agg_ps", bufs=2, space="PSUM") as agg_ps,
            tc.tile_pool(name="wk_ps", bufs=2, space="PSUM") as wk_ps,
        ):
            # ---- tiny junk tile: PE p-state warmup + ACT table warm ----
            junk = consts.tile([P, P], BF16, tag="junk")
            nc.vector.memset(junk[:], 0.0)
            warm = consts.tile([1, 1], F32, tag="warm")
            nc.scalar.activation(warm[:], junk[0:1, 0:1], EXP)

            # ---- DMA issue order (SP and ACT HWDGE queues).
            # Transfers serialize on the DMA engines in arrival order, so
            # the first-scores chain (wq, xT0, wk, sT q0) leads.
            xT_sb = bigin.tile([P, c_ch, N_SEQ], BF16, tag="xT")
            sT_sb = bigin.tile([P, c_ch, m_pad], BF16, tag="sT")
            wq_sb = consts.tile([P, c_ch, P], BF16, tag="wq")
            wk_sb = consts.tile([P, c_ch, P], BF16, tag="wk")
            wvmi_sb = consts.tile([P, DIM + DIM + m_ch], BF16, tag="wvmi")

            # sT load quarter, rounded up to a multiple of 128
            M_Q = max(P, ((m_pad // 4 + P - 1) // P) * P)

            def load_st(lo, hi, eng):
                msl = bass.ds(lo, hi - lo)
                eng.dma_start(sT_sb[:, :, msl], sT.ap()[:, :, msl])

            def load_xt(nt, eng):
                nsl = bass.ds(nt * NT_W, NT_W)
                eng.dma_start(xT_sb[:, :, nsl], xT.ap()[:, :, nsl])

            # All on SP, in critical-path priority order (transfers serialize
            # on the DMA engines in arrival order): the first-scores chain
            # xT0 -> wq -> wk -> sT q0 leads, bulk xT tiles trail.
            load_xt(0, nc.sync)
            nc.sync.dma_start(wq_sb[:], wq.ap())
            nc.sync.dma_start(wk_sb[:], wk.ap())
            load_st(0, min(M_Q, m_pad), nc.sync)
            for q in range(1, 4):
                lo = q * M_Q
                if lo < m_pad:
                    load_st(lo, min(lo + M_Q, m_pad), nc.sync)
            # wvmi (V/merge weights + indicator) is first needed by the agg
            # of (0, h0, g0), well after the sT quarters that pace the exps
            nc.sync.dma_start(wvmi_sb[:], wvmi.ap())
            load_xt(1, nc.sync)
            load_xt(2, nc.sync)
            load_xt(3, nc.sync)

            # identity for the PE transpose; zero tile for PSUM bank resets
            ident = consts.tile([P, P], BF16, tag="ident")
            make_identity(nc, ident[:])
            z260 = consts.tile([P, 4 * 65], BF16, tag="z260")
            nc.vector.memset(z260[:], 0.0)

            # PE warm-up: junk bf16 matmuls so the PE clock-gate is ramping
            # while the first DMAs land (~full rate from ~3.1us on).
            for _ in range(28):
                wps = wk_ps.tile([P, NT_W], F32, tag="w")
                nc.tensor.matmul(wps[:, 0:P], junk[:], junk[:],
                                 start=True, stop=True)

            # views into wvmi
            wv_view = wvmi_sb[:, 0:DIM]            # [P, 512] -> co slices
            wm_view = wvmi_sb[:, DIM:2 * DIM]      # [P, 512]
            ind_view = wvmi_sb[:, 2 * DIM:2 * DIM + m_ch]

            qt_sb = proj.tile([P, N_SEQ], BF16, tag="qt")
            kt_sb = proj.tile([P, m_pad], BF16, tag="kt")
            v_sb = proj.tile([P, m_ch, 130], BF16, tag="v")

            def emit_qt(nt):
                nsl = bass.ds(nt * NT_W, NT_W)
                ps = wk_ps.tile([P, NT_W], F32, tag="w")
                for co in range(c_ch):
                    nc.tensor.matmul(ps[:], wq_sb[:, co, :], xT_sb[:, co, nsl],
                                     start=(co == 0), stop=(co == c_ch - 1))
                nc.vector.tensor_copy(qt_sb[:, nsl], ps[:])

            def emit_kt(lo, hi):
                msl = bass.ds(lo, hi - lo)
                ps = wk_ps.tile([P, NT_W], F32, tag="w")
                for co in range(c_ch):
                    nc.tensor.matmul(ps[:, 0:hi - lo], wk_sb[:, co, :],
                                     sT_sb[:, co, msl],
                                     start=(co == 0), stop=(co == c_ch - 1))
                nc.vector.tensor_copy(kt_sb[:, msl], ps[:, 0:hi - lo])

            def emit_v(mc):
                msl = bass.ds(mc * P, P)
                ps = wk_ps.tile([P, NT_W], F32, tag="w")
                for co in range(c_ch):
                    nc.tensor.matmul(
                        ps[:, 0:P], sT_sb[:, co, msl],
                        wv_view[:, bass.ds(co * P, P)],
                        start=(co == 0), stop=(co == c_ch - 1))
                # v_sb[:, mc, 0:64] = ps[:, 0:64]; v_sb[:, mc, 65:129] = ps[:, 64:128]
                dst = v_sb[:, mc, :].rearrange("p (two dh) -> p two dh", two=2)
                src = ps[:, 0:P].rearrange("p (two dh) -> p two dh", two=2)
                # ACT has slack between the DMA-paced first exps; DVE must
                # stay clear for the kt quarter copies that gate the scores
                nc.scalar.copy(dst[:, :, 0:64], src)

            def emit_ind():
                nc.vector.tensor_copy(v_sb[:, :, 64], ind_view)
                nc.vector.tensor_copy(v_sb[:, :, 129], ind_view)

            # ------------- main interleaved stream -------------
            # ACT is paced by score groups; all other PE work drains from a
            # FIFO backlog in the slack between score groups.
            backlog = []  # list of 0-arg emitters

            slot = [0]   # global (nt, h, g) slot counter for backlog gating

            def pump(budget):
                # emit backlog items; `budget` ~ PE columns worth of slack.
                # Items carry a min_slot so cross-engine chains (norm ->
                # transpose -> merge) get a slot of latency between stages
                # instead of stalling the in-order PE.
                while backlog and budget > 0:
                    fn, cost, min_slot = backlog[0]
                    if min_slot > slot[0]:
                        break
                    if cost > budget and budget < 1200:
                        break
                    backlog.pop(0)
                    fn()
                    budget -= cost
                return budget

            state = {}   # (nt, h) -> A psum tile ; nt -> aggT etc.

            def scores_exp(nt, h, g, e_t, n0=0, nw=NT_W):
                nsl = bass.ds(nt * NT_W + n0, nw)
                hsl = bass.ds(64 * h, 64)
                mcs = [mc for mc in (2 * g, 2 * g + 1) if mc < m_ch]
                sp = sc_ps.tile([P, 2, NT_W], F32, tag="sc")
                for j, mc in enumerate(mcs):
                    nc.tensor.matmul(sp[:, j, 0:nw],
                                     kt_sb[hsl, bass.ds(mc * P, P)],
                                     qt_sb[hsl, nsl],
                                     start=True, stop=True)
                nc.scalar.activation(e_t[:, 2 * g:2 * g + len(mcs),
                                         n0:n0 + nw],
                                     sp[:, 0:len(mcs), 0:nw],
                                     EXP, scale=0.125)

            def agg_zero(A_h):
                # start=True resets the full PSUM bank, so interleaved
                # accumulation groups in one bank need a single whole-tile
                # start; all real agg matmuls then accumulate (start=False).
                nc.tensor.matmul(A_h[:, :, :], junk[:], z260[:, 0:NC * 65],
                                 start=True, stop=False, skip_group_check=True)

            def agg_group(nt, h, g, e_t, A_h, c0=0, cn=NC):
                if g == 0 and c0 == 0:
                    agg_zero(A_h)
                for mc in (2 * g, 2 * g + 1):
                    if mc >= m_ch:
                        continue
                    for ncc in range(c0, c0 + cn):
                        nc.tensor.matmul(
                            A_h[:, ncc, :],
                            e_t[:, mc, bass.ds(ncc * P, P)],
                            v_sb[:, mc, bass.ds(65 * h, 65)],
                            start=False, stop=(mc == m_ch - 1),
                            skip_group_check=True)

            def norm_recip(A_h, c0=0, cn=NC):
                r_sb = small.tile([P, NC], F32, tag="r")
                nc.vector.reciprocal(r_sb[:, c0:c0 + cn],
                                     A_h[:, c0:c0 + cn, 64])
                return r_sb

            def norm_mul(h, A_h, aggT_t, r_sb, ncc):
                nc.vector.tensor_scalar_mul(
                    aggT_t[:, ncc, bass.ds(64 * h, 64)],
                    A_h[:, ncc, 0:64],
                    r_sb[:, bass.ds(ncc, 1)])

            def norm(nt, h, A_h, aggT_t):
                r_sb = norm_recip(A_h)
                for ncc in range(NC):
                    norm_mul(h, A_h, aggT_t, r_sb, ncc)

            def transpose_chunk(aggT_t, agg_t, ncc, copy_eng="dve"):
                # each transpose start-resets its PSUM bank, so one rotating
                # tile per chunk with an immediate copy out
                tr_t = wk_ps.tile([P, P], BF16, tag="w")
                nc.tensor.transpose(tr_t[:], aggT_t[:, ncc, :], ident[:])
                if copy_eng == "act":
                    nc.scalar.copy(agg_t[:, ncc, :], tr_t[:])
                else:
                    nc.vector.tensor_copy(agg_t[:, ncc, :], tr_t[:])

            def merge(nt, ncc, agg_t, o_t, copy_eng="dve", pool=None):
                mp = (pool or wk_ps).tile([P, NT_W], F32, tag="w" if pool is None else "A",
                                          name="mp")
                nc.tensor.matmul(mp[:], agg_t[:, ncc, :], wm_view,
                                 start=True, stop=True)
                if copy_eng == "act":
                    nc.scalar.copy(o_t[:, ncc, :], mp[:])
                else:
                    nc.vector.tensor_copy(o_t[:, ncc, :], mp[:])

            def out_dma(nt, ncc, o_t):
                n0 = nt * NT_W + ncc * P
                nc.sync.dma_start(out.ap()[bass.ds(n0, P), :], o_t[:, ncc, :])

            def alloc_finalize(nt):
                state[("fin", nt)] = (
                    aggt.tile([P, NC, P], BF16, tag="aggT", name="aggT_t"),
                    aggsb.tile([P, NC, P], BF16, tag="agg", name="agg_t"),
                    outp.tile([P, NC, DIM], BF16, tag="o", name="o_t"),
                )

            def queue_finalize(nt, s):
                aggT_t, agg_t, o_t = state[("fin", nt)]
                A_h1 = state.pop((nt, 1))
                backlog.append((lambda: norm(nt, 1, A_h1, aggT_t), 0, s))
                for ncc in range(NC):
                    backlog.append(
                        (lambda ncc=ncc: transpose_chunk(aggT_t, agg_t, ncc),
                         128, s + 2))
                for ncc in range(NC):
                    backlog.append(
                        (lambda ncc=ncc: merge(nt, ncc, agg_t, o_t),
                         512, s + 3))
                    backlog.append(
                        (lambda ncc=ncc: out_dma(nt, ncc, o_t), 0, s + 3))

            def finalize_part(nt, A_h1, c0, cn, inline=False, s=0):
                # per-chunk pipeline: DVE norm muls, ACT transpose copies,
                # ACT/DVE alternate on output copies, DMA per chunk
                aggT_t, agg_t, o_t = state[("fin", nt)]

                def fin_norm():
                    r_sb = norm_recip(A_h1, c0, cn)
                    for ncc in range(c0, c0 + cn):
                        norm_mul(1, A_h1, aggT_t, r_sb, ncc)

                def fin_tr():
                    for ncc in range(c0, c0 + cn):
                        transpose_chunk(aggT_t, agg_t, ncc, copy_eng="act")

                def fin_merge(ncc):
                    # A banks are free after the norms for the very last
                    # part; spread merges so they issue back-to-back
                    merge(nt, ncc, agg_t, o_t,
                          copy_eng="dve" if ncc % 2 == 0 else "act",
                          pool=agg_ps if (inline and ncc % 2 == 0) else None)
                    out_dma(nt, ncc, o_t)

                if inline:
                    fin_norm()
                    fin_tr()
                    for ncc in range(c0, c0 + cn):
                        fin_merge(ncc)
                else:
                    backlog.append((fin_norm, 0, s + 1))
                    backlog.append((fin_tr, 256, s + 2))
                    for ncc in range(c0, c0 + cn):
                        backlog.append(
                            (lambda ncc=ncc: fin_merge(ncc), 512, s + 3))

            # prologue: QT0 + KT quarters feed the first score groups
            emit_qt(0)
            kt_edge = 0

            def emit_kt_quarter():
                nonlocal kt_edge
                if kt_edge < m_pad:
                    hi = min(kt_edge + M_Q, m_pad)
                    emit_kt(kt_edge, hi)
                    kt_edge = hi

            emit_kt_quarter()

            first = True
            for nt in range(n_tiles):
                alloc_finalize(nt)
                for h in (0, 1):
                    e_t = exps.tile([P, m_ch, NT_W], BF16, tag="e")
                    A_h = agg_ps.tile([P, NC, 65], F32, tag="A")
                    state[(nt, h)] = A_h
                    if nt == n_tiles - 1 and h == 1:
                        # last head: two half-width passes over n so the
                        # first half's finalize overlaps the second half's
                        # exps, halving the exposed tail
                        ca, cb = NC // 2, NC - NC // 2
                        for g in range(n_groups):
                            scores_exp(nt, h, g, e_t, 0, ca * P)
                            backlog.append(
                                (lambda g=g: agg_group(nt, 1, g, e_t, A_h,
                                                       0, ca), 260,
                                 slot[0] + 2))
                            slot[0] += 1
                            pump(700)
                        finalize_part(nt, A_h, 0, ca, s=slot[0])
                        for g in range(n_groups):
                            scores_exp(nt, h, g, e_t, ca * P, cb * P)
                            backlog.append(
                                (lambda g=g: agg_group(nt, 1, g, e_t, A_h,
                                                       ca, cb), 260,
                                 slot[0] + 2))
                            slot[0] += 1
                            pump(700)
                        while backlog:
                            fn, _, _ = backlog.pop(0)
                            fn()
                        finalize_part(nt, A_h, ca, cb, inline=True)
                        continue
                    for g in range(n_groups):
                        if first:
                            # KT chunks this group reads must be emitted
                            # before the score matmuls (in-order PE)
                            while kt_edge < min((2 * g + 2) * P, m_pad):
                                emit_kt_quarter()
                        scores_exp(nt, h, g, e_t)
                        if first:
                            # prefetch the next KT quarter; stream V in the
                            # slack of the first score groups
                            emit_kt_quarter()
                            if g == 0:
                                backlog.append((emit_ind, 0, 0))
                                # spread V emission: pair mc//2 lands just
                                # before the agg group that first reads it
                                for mc in range(m_ch):
                                    backlog.append(
                                        (lambda mc=mc: emit_v(mc), 128,
                                         mc // 2 + 1))
                        backlog.append(
                            (lambda nt=nt, h=h, g=g, e_t=e_t, A_h=A_h:
                             agg_group(nt, h, g, e_t, A_h), 520,
                             slot[0] + 2))
                        # QT for the next tile must be emitted before that
                        # tile's first score group reads qt_sb (in-order PE)
                        if h == 1 and g == 1 and nt + 1 < n_tiles:
                            emit_qt(nt + 1)
                        # skip pumping right before the next head's first
                        # score group so the score stream never pauses on
                        # the in-order PE; catch up with a double pump after
                        slot[0] += 1
                        if g == n_groups - 1:
                            budget = 0
                        elif g == 0:
                            budget = 2400
                        else:
                            budget = 1200
                        pump(budget if not first else 700)
                    if h == 0:
                        # norm h0 once its last agg group has drained
                        aggT_t = state[("fin", nt)][0]
                        A_h0 = state.pop((nt, 0))
                        backlog.append(
                            (lambda nt=nt, A_h0=A_h0, aggT_t=aggT_t:
                             norm(nt, 0, A_h0, aggT_t), 0, slot[0] + 2))
                    first = False
                if nt < n_tiles - 1:
                    queue_finalize(nt, slot[0])

            # defensive drain (the last-head path drains inline)
            while backlog:
                fn, _, _ = backlog.pop(0)
                fn()

    nc.compile()
    return nc


def shard_inputs(x, source, source_mask, Wq, Wk, Wv, Wm):
    """Build the 8 per-core input dicts. Returns (in_maps, m_pad)."""
    B = x.shape[0]
    valid = [np.flatnonzero(source_mask[b]) for b in range(B)]
    m_pad = max(P, int(math.ceil(max(len(v) for v in valid) / P)) * P)
    m_ch = m_pad // P

    def pack_k_major(w_part):
        # w_part [128 out-dims, 512 in-dims] -> [p, co, d] with k=co*128+p
        return np.ascontiguousarray(
            w_part.T.reshape(4, P, P).transpose(1, 0, 2)).astype(NP_BF16)

    per_batch = []
    for b in range(B):
        v = valid[b]
        xTb = np.ascontiguousarray(
            x[b].T.reshape(4, P, N_SEQ).transpose(1, 0, 2)).astype(NP_BF16)
        sc = np.zeros((m_pad, DIM), np.float32)
        sc[: len(v)] = source[b][v]
        sTb = np.ascontiguousarray(
            sc.T.reshape(4, P, m_pad).transpose(1, 0, 2)).astype(NP_BF16)
        indb = np.zeros((m_pad,), np.float32)
        indb[: len(v)] = 1.0
        ind_pm = np.ascontiguousarray(indb.reshape(m_ch, P).T)  # [p, mc]
        per_batch.append((xTb, sTb, ind_pm))

    in_maps = []
    for core in range(8):
        b, hp = divmod(core, 4)
        rows = slice(hp * P, (hp + 1) * P)
        xTb, sTb, ind_pm = per_batch[b]
        wv_p = pack_k_major(np.asarray(Wv)[rows])          # [p, co, d]
        wm_p = np.asarray(Wm)[:, rows].T.astype(np.float32)  # [p=128, 512]
        wvmi = np.concatenate(
            [wv_p.reshape(P, DIM), wm_p, ind_pm], axis=1).astype(NP_BF16)
        in_maps.append({
            "xT": xTb,
            "sT": sTb,
            "wq": pack_k_major(np.asarray(Wq)[rows]),
            "wk": pack_k_major(np.asarray(Wk)[rows]),
            "wvmi": np.ascontiguousarray(wvmi),
        })
    return in_maps, m_pad


_CACHE = {}


def _get_nc(m_pad):
    if m_pad not in _CACHE:
        _CACHE[m_pad] = build_bass(m_pad)
    return _CACHE[m_pad]


def kernel(x, source, source_mask, Wq, bq, Wk, bk, Wv, bv, Wm, bm,
           _trace=False, **run_kwargs):
    x = np.asarray(x)
    source = np.asarray(source)
    source_mask = np.asarray(source_mask)
    Wq, Wk, Wv, Wm = (np.asarray(a) for a in (Wq, Wk, Wv, Wm))
    bq, bk, bv, bm = (np.asarray(a) for a in (bq, bk, bv, bm))

    # Biases in this problem are structurally zero; the device program folds
    # none, so guard against surprises.
    assert not (np.any(bq) or np.any(bk) or np.any(bv)), \
        "nonzero q/k/v biases not supported by this kernel build"

    in_maps, m_pad = shard_inputs(x, source, source_mask, Wq, Wk, Wv, Wm)
    nc = _get_nc(m_pad)
    res = run_bass_kernel_spmd(nc, in_maps, core_ids=list(range(8)),
                               trace=_trace, **run_kwargs)

    B, N, D = x.shape
    outv = np.zeros((B, N, D), np.float32)
    for core in range(8):
        b = core // 4
        outv[b] += np.asarray(res.results[core]["out"], dtype=np.float32)
    outv += bm.astype(np.float32)
    if _trace:
        kernel._last_results = res
    return outv


# revision 44
# speedup vs baseline: 1.0015x; 1.0015x over previous
"""Trainium2 Bass kernel for the cross-attention block.

Sharding: B=2 batches x 4 head-pairs = 8 cores. Core i handles batch i//4 and
heads (2*(i%4), 2*(i%4)+1). Each core computes its heads' full NxM attention
and a partial merge projection; the host sums the 4 partials per batch.

Host-side preprocessing (cheap numpy):
  - masked source rows are filtered out entirely (softmax over m is
    permutation-invariant; masked rows contribute nothing), padded to a
    multiple of 128 with zero rows + a 0 indicator so pads drop out of both
    numerator and denominator.
  - x / source / weights are pre-transposed AND cast to bf16 so the device
    never transposes and DMA bytes are halved.

Device pipeline per core (all matmuls bf16 in / fp32 psum out):
  QT[hd=128, n]  = wq.T @ xT          (4 c-chunks accumulated)
  KT[hd=128, m]  = wk.T @ sT
  V_aug[m, 130]  = sT.T @ wv | ind    (cols 64/129 hold the valid-row mask)
  per (n-tile 512, head, group of 2 m-chunks):
      S[m=128, n=512] = KT_h-chunk.T @ QT_h     (K = 64 head dims)
      e = Exp(0.125 * S)   -> bf16 SBUF         (ScalarE, the bottleneck)
  per (head, n-chunk 128): A[n, 65] = sum_mc e[mc].T @ V_aug_h[mc]
      col 64 = softmax denominator (per-partition scalar!)
      r = 1/A[:,64] (DVE reciprocal), aggT[n, d] = A[:,0:64] * r  (DVE)
  transpose aggT[n, 128] -> agg[d=128, n] per n-chunk (PE, identity matmul)
  out[n, 512] = agg.T @ wm  -> SBUF copy (Pool) -> DMA per n-chunk

Engine budget (TimelineSim cost model): ACT = 32 exp x ~1.04us = 33.2us is
the floor; PE ~= 32us hides under it; DVE ~14us; Pool ~13us; DMA ~20us.
"""

import math
import sys

import numpy as np

try:
    import concourse.bass as bass
except ImportError:  # pragma: no cover - fresh grading dir safety
    sys.path.insert(0, "/opt/trn_rl_repo")
    import concourse.bass as bass

import concourse.mybir as mybir
from concourse import bacc
import concourse.tile as tile
from concourse.bass_utils import run_bass_kernel_spmd
from concourse.masks import make_identity

F32 = mybir.dt.float32
BF16 = mybir.dt.bfloat16
NP_BF16 = mybir.dt.np(mybir.dt.bfloat16)
EXP = mybir.ActivationFunctionType.Exp

DIM = 512
N_SEQ = 2048
NT_W = 512          # n-tile width for the score/exp stream
P = 128


def build_bass(m_pad: int):
    """Build the per-core Bass program. m_pad: padded valid-source length."""
    m_ch = m_pad // P            # m chunks of 128
    n_groups = (m_ch + 1) // 2   # exp groups of 2 m-chunks
    n_tiles = N_SEQ // NT_W      # 4
    c_ch = DIM // P              # 4 contraction chunks for projections
    NC = NT_W // P               # n-chunks of 128 per n-tile

    nc = bacc.Bacc("TRN2", target_bir_lowering=False, debug=False,
                   num_devices=8)

    xT = nc.dram_tensor("xT", [P, c_ch, N_SEQ], BF16, kind="ExternalInput")
    sT = nc.dram_tensor("sT", [P, c_ch, m_pad], BF16, kind="ExternalInput")
    wq = nc.dram_tensor("wq", [P, c_ch, P], BF16, kind="ExternalInput")
    wk = nc.dram_tensor("wk", [P, c_ch, P], BF16, kind="ExternalInput")
    # wvmi: wv [P, 512] | wm [P, 512] | ind [P, m_ch]
    wvmi = nc.dram_tensor("wvmi", [P, DIM + DIM + m_ch], BF16,
                          kind="ExternalInput")
    # bf16 output halves the out-DMA; the host sums the 4 partials in fp32
    out = nc.dram_tensor("out", [N_SEQ, DIM], BF16, kind="ExternalOutput")

    with tile.TileContext(nc) as tc:
        with (
            tc.tile_pool(name="consts", bufs=1) as consts,
            tc.tile_pool(name="bigin", bufs=1) as bigin,
            tc.tile_pool(name="proj", bufs=1) as proj,
            tc.tile_pool(name="exps", bufs=3) as exps,
            tc.tile_pool(name="aggt", bufs=2) as aggt,
            tc.tile_pool(name="aggsb", bufs=2) as aggsb,
            tc.tile_pool(name="outp", bufs=2) as outp,
            tc.tile_pool(name="small", bufs=4) as small,
            tc.tile_pool(name="sc_ps", bufs=2, space="PSUM") as sc_ps,
            tc.tile_pool(name="agg_ps", bufs=2, space="PSUM") as agg_ps,
            tc.tile_pool(name="wk_ps", bufs=2, space="PSUM") as wk_ps,
        ):
            # ---- tiny junk tile: PE p-state warmup + ACT table warm ----
            junk = consts.tile([P, P], BF16, tag="junk")
            nc.vector.memset(junk[:], 0.0)
            warm = consts.tile([1, 1], F32, tag="warm")
            nc.scalar.activation(warm[:], junk[0:1, 0:1], EXP)

            # ---- DMA issue order (SP and ACT HWDGE queues).
            # Transfers serialize on the DMA engines in arrival order, so
            # the first-scores chain (wq, xT0, wk, sT q0) leads.
            xT_sb = bigin.tile([P, c_ch, N_SEQ], BF16, tag="xT")
            sT_sb = bigin.tile([P, c_ch, m_pad], BF16, tag="sT")
            wq_sb = consts.tile([P, c_ch, P], BF16, tag="wq")
            wk_sb = consts.tile([P, c_ch, P], BF16, tag="wk")
            wvmi_sb = consts.tile([P, DIM + DIM + m_ch], BF16, tag="wvmi")

            # sT load quarter, rounded up to a multiple of 128
            M_Q = max(P, ((m_pad // 4 + P - 1) // P) * P)

            def load_st(lo, hi, eng):
                msl = bass.ds(lo, hi - lo)
                eng.dma_start(sT_sb[:, :, msl], sT.ap()[:, :, msl])

            def load_xt(nt, eng):
                nsl = bass.ds(nt * NT_W, NT_W)
                eng.dma_start(xT_sb[:, :, nsl], xT.ap()[:, :, nsl])

            # All on SP, in critical-path priority order (transfers serialize
            # on the DMA engines in arrival order): the first-scores chain
            # xT0 -> wq -> wk -> sT q0 leads, bulk xT tiles trail.
            load_xt(0, nc.sync)
            nc.sync.dma_start(wq_sb[:], wq.ap())
            nc.sync.dma_start(wk_sb[:], wk.ap())
            load_st(0, min(M_Q, m_pad), nc.sync)
            for q in range(1, 4):
                lo = q * M_Q
                if lo < m_pad:
                    load_st(lo, min(lo + M_Q, m_pad), nc.sync)
            # wvmi (V/merge weights + indicator) is first needed by the agg
            # of (0, h0, g0), well after the sT quarters that pace the exps
            nc.sync.dma_start(wvmi_sb[:], wvmi.ap())
            load_xt(1, nc.sync)
            load_xt(2, nc.sync)
            load_xt(3, nc.sync)

            # identity for the PE transpose; zero tile for PSUM bank resets
            ident = consts.tile([P, P], BF16, tag="ident")
            make_identity(nc, ident[:])
            z260 = consts.tile([P, 4 * 65], BF16, tag="z260")
            nc.vector.memset(z260[:], 0.0)

            # PE warm-up: junk bf16 matmuls so the PE clock-gate is ramping
            # while the first DMAs land (~full rate from ~3.1us on).
            for _ in range(28):
                wps = wk_ps.tile([P, NT_W], F32, tag="w")
                nc.tensor.matmul(wps[:, 0:P], junk[:], junk[:],
                                 start=True, stop=True)

            # views into wvmi
            wv_view = wvmi_sb[:, 0:DIM]            # [P, 512] -> co slices
            wm_view = wvmi_sb[:, DIM:2 * DIM]      # [P, 512]
            ind_view = wvmi_sb[:, 2 * DIM:2 * DIM + m_ch]

            qt_sb = proj.tile([P, N_SEQ], BF16, tag="qt")
            kt_sb = proj.tile([P, m_pad], BF16, tag="kt")
            v_sb = proj.tile([P, m_ch, 130], BF16, tag="v")

            def emit_qt(nt):
                nsl = bass.ds(nt * NT_W, NT_W)
                ps = wk_ps.tile([P, NT_W], F32, tag="w")
                for co in range(c_ch):
                    nc.tensor.matmul(ps[:], wq_sb[:, co, :], xT_sb[:, co, nsl],
                                     start=(co == 0), stop=(co == c_ch - 1))
                nc.vector.tensor_copy(qt_sb[:, nsl], ps[:])

            def emit_kt(lo, hi):
                msl = bass.ds(lo, hi - lo)
                ps = wk_ps.tile([P, NT_W], F32, tag="w")
                for co in range(c_ch):
                    nc.tensor.matmul(ps[:, 0:hi - lo], wk_sb[:, co, :],
                                     sT_sb[:, co, msl],
                                     start=(co == 0), stop=(co == c_ch - 1))
                nc.vector.tensor_copy(kt_sb[:, msl], ps[:, 0:hi - lo])

            def emit_v(mc):
                msl = bass.ds(mc * P, P)
                ps = wk_ps.tile([P, NT_W], F32, tag="w")
                for co in range(c_ch):
                    nc.tensor.matmul(
                        ps[:, 0:P], sT_sb[:, co, msl],
                        wv_view[:, bass.ds(co * P, P)],
                        start=(co == 0), stop=(co == c_ch - 1))
                # v_sb[:, mc, 0:64] = ps[:, 0:64]; v_sb[:, mc, 65:129] = ps[:, 64:128]
                dst = v_sb[:, mc, :].rearrange("p (two dh) -> p two dh", two=2)
                src = ps[:, 0:P].rearrange("p (two dh) -> p two dh", two=2)
                # ACT has slack between the DMA-paced first exps; DVE must
                # stay clear for the kt quarter copies that gate the scores
                nc.scalar.copy(dst[:, :, 0:64], src)

            def emit_ind():
                nc.vector.tensor_copy(v_sb[:, :, 64], ind_view)
                nc.vector.tensor_copy(v_sb[:, :, 129], ind_view)

            # ------------- main interleaved stream -------------
            # ACT is paced by score groups; all other PE work drains from a
            # FIFO backlog in the slack between score groups.
            backlog = []  # list of 0-arg emitters

            slot = [0]   # global (nt, h, g) slot counter for backlog gating

            def pump(budget):
                # emit backlog items; `budget` ~ PE columns worth of slack.
                # Items carry a min_slot so cross-engine chains (norm ->
                # transpose -> merge) get a slot of latency between stages
                # instead of stalling the in-order PE.
                while backlog and budget > 0:
                    fn, cost, min_slot = backlog[0]
                    if min_slot > slot[0]:
                        break
                    if cost > budget and budget < 1200:
                        break
                    backlog.pop(0)
                    fn()
                    budget -= cost
                return budget

            state = {}   # (nt, h) -> A psum tile ; nt -> aggT etc.

            def scores_exp(nt, h, g, e_t, n0=0, nw=NT_W):
                nsl = bass.ds(nt * NT_W + n0, nw)
                hsl = bass.ds(64 * h, 64)
                mcs = [mc for mc in (2 * g, 2 * g + 1) if mc < m_ch]
                sp = sc_ps.tile([P, 2, NT_W], F32, tag="sc")
                for j, mc in enumerate(mcs):
                    nc.tensor.matmul(sp[:, j, 0:nw],
                                     kt_sb[hsl, bass.ds(mc * P, P)],
                                     qt_sb[hsl, nsl],
                                     start=True, stop=True)
                nc.scalar.activation(e_t[:, 2 * g:2 * g + len(mcs),
                                         n0:n0 + nw],
                                     sp[:, 0:len(mcs), 0:nw],
                                     EXP, scale=0.125)

            def agg_zero(A_h):
                # start=True resets the full PSUM bank, so interleaved
                # accumulation groups in one bank need a single whole-tile
                # start; all real agg matmuls then accumulate (start=False).
                nc.tensor.matmul(A_h[:, :, :], junk[:], z260[:, 0:NC * 65],
                                 start=True, stop=False, skip_group_check=True)

            def agg_group(nt, h, g, e_t, A_h, c0=0, cn=NC):
                if g == 0 and c0 == 0:
                    agg_zero(A_h)
                for mc in (2 * g, 2 * g + 1):
                    if mc >= m_ch:
                        continue
                    for ncc in range(c0, c0 + cn):
                        nc.tensor.matmul(
                            A_h[:, ncc, :],
                            e_t[:, mc, bass.ds(ncc * P, P)],
                            v_sb[:, mc, bass.ds(65 * h, 65)],
                            start=False, stop=(mc == m_ch - 1),
                            skip_group_check=True)

            def norm_recip(A_h, c0=0, cn=NC):
                r_sb = small.tile([P, NC], F32, tag="r")
                nc.vector.reciprocal(r_sb[:, c0:c0 + cn],
                                     A_h[:, c0:c0 + cn, 64])
                return r_sb

            def norm_mul(h, A_h, aggT_t, r_sb, ncc):
                nc.vector.tensor_scalar_mul(
                    aggT_t[:, ncc, bass.ds(64 * h, 64)],
                    A_h[:, ncc, 0:64],
                    r_sb[:, bass.ds(ncc, 1)])

            def norm(nt, h, A_h, aggT_t):
                r_sb = norm_recip(A_h)
                for ncc in range(NC):
                    norm_mul(h, A_h, aggT_t, r_sb, ncc)

            def transpose_chunk(aggT_t, agg_t, ncc, copy_eng="dve"):
                # each transpose start-resets its PSUM bank, so one rotating
                # tile per chunk with an immediate copy out
                tr_t = wk_ps.tile([P, P], BF16, tag="w")
                nc.tensor.transpose(tr_t[:], aggT_t[:, ncc, :], ident[:])
                if copy_eng == "act":
                    nc.scalar.copy(agg_t[:, ncc, :], tr_t[:])
                else:
                    nc.vector.tensor_copy(agg_t[:, ncc, :], tr_t[:])

            def merge(nt, ncc, agg_t, o_t, copy_eng="dve", pool=None):
                mp = (pool or wk_ps).tile([P, NT_W], F32, tag="w" if pool is None else "A",
                                          name="mp")
                nc.tensor.matmul(mp[:], agg_t[:, ncc, :], wm_view,
                                 start=True, stop=True)
                if copy_eng == "act":
                    nc.scalar.copy(o_t[:, ncc, :], mp[:])
                else:
                    nc.vector.tensor_copy(o_t[:, ncc, :], mp[:])

            def out_dma(nt, ncc, o_t):
                n0 = nt * NT_W + ncc * P
                nc.sync.dma_start(out.ap()[bass.ds(n0, P), :], o_t[:, ncc, :])

            def alloc_finalize(nt):
                state[("fin", nt)] = (
                    aggt.tile([P, NC, P], BF16, tag="aggT", name="aggT_t"),
                    aggsb.tile([P, NC, P], BF16, tag="agg", name="agg_t"),
                    outp.tile([P, NC, DIM], BF16, tag="o", name="o_t"),
                )

            def queue_finalize(nt, s):
                aggT_t, agg_t, o_t = state[("fin", nt)]
                A_h1 = state.pop((nt, 1))
                backlog.append((lambda: norm(nt, 1, A_h1, aggT_t), 0, s))
                for ncc in range(NC):
                    backlog.append(
                        (lambda ncc=ncc: transpose_chunk(aggT_t, agg_t, ncc),
                         128, s + 2))
                for ncc in range(NC):
                    backlog.append(
                        (lambda ncc=ncc: merge(nt, ncc, agg_t, o_t),
                         512, s + 3))
                    backlog.append(
                        (lambda ncc=ncc: out_dma(nt, ncc, o_t), 0, s + 3))

            def finalize_part(nt, A_h1, c0, cn, inline=False, s=0):
                # per-chunk pipeline: DVE norm muls, ACT transpose copies,
                # ACT/DVE alternate on output copies, DMA per chunk
                aggT_t, agg_t, o_t = state[("fin", nt)]

                def fin_norm():
                    r_sb = norm_recip(A_h1, c0, cn)
                    for ncc in range(c0, c0 + cn):
                        norm_mul(1, A_h1, aggT_t, r_sb, ncc)

                def fin_tr():
                    for ncc in range(c0, c0 + cn):
                        # ACT still runs the second half's exps during the
                        # non-inline part: keep its copies on DVE then
                        transpose_chunk(aggT_t, agg_t, ncc,
                                        copy_eng="act" if inline else "dve")

                def fin_merge(ncc):
                    # A banks are free after the norms for the very last
                    # part; spread merges so they issue back-to-back
                    merge(nt, ncc, agg_t, o_t,
                          copy_eng=("dve" if ncc % 2 == 0 else "act")
                          if inline else "dve",
                          pool=agg_ps if (inline and ncc % 2 == 0) else None)
                    out_dma(nt, ncc, o_t)

                if inline:
                    fin_norm()
                    fin_tr()
                    for ncc in range(c0, c0 + cn):
                        fin_merge(ncc)
                else:
                    backlog.append((fin_norm, 0, s + 1))
                    backlog.append((fin_tr, 256, s + 2))
                    for ncc in range(c0, c0 + cn):
                        backlog.append(
                            (lambda ncc=ncc: fin_merge(ncc), 512, s + 3))

            # prologue: QT0 + KT quarters feed the first score groups
            emit_qt(0)
            kt_edge = 0

            def emit_kt_quarter():
                nonlocal kt_edge
                if kt_edge < m_pad:
                    hi = min(kt_edge + M_Q, m_pad)
                    emit_kt(kt_edge, hi)
                    kt_edge = hi

            emit_kt_quarter()

            first = True
            for nt in range(n_tiles):
                alloc_finalize(nt)
                for h in (0, 1):
                    e_t = exps.tile([P, m_ch, NT_W], BF16, tag="e")
                    A_h = agg_ps.tile([P, NC, 65], F32, tag="A")
                    state[(nt, h)] = A_h
                    if nt == n_tiles - 1 and h == 1:
                        # last head: two half-width passes over n so the
                        # first half's finalize overlaps the second half's
                        # exps, halving the exposed tail
                        ca, cb = NC // 2, NC - NC // 2
                        for g in range(n_groups):
                            scores_exp(nt, h, g, e_t, 0, ca * P)
                            backlog.append(
                                (lambda g=g: agg_group(nt, 1, g, e_t, A_h,
                                                       0, ca), 260,
                                 slot[0] + 2))
                            slot[0] += 1
                            pump(700)
                        finalize_part(nt, A_h, 0, ca, s=slot[0])
                        for g in range(n_groups):
                            scores_exp(nt, h, g, e_t, ca * P, cb * P)
                            backlog.append(
                                (lambda g=g: agg_group(nt, 1, g, e_t, A_h,
                                                       ca, cb), 260,
                                 slot[0] + 2))
                            slot[0] += 1
                            pump(700)
                        while backlog:
                            fn, _, _ = backlog.pop(0)
                            fn()
                        finalize_part(nt, A_h, ca, cb, inline=True)
                        continue
                    for g in range(n_groups):
                        if first:
                            # KT chunks this group reads must be emitted
                            # before the score matmuls (in-order PE)
                            while kt_edge < min((2 * g + 2) * P, m_pad):
                                emit_kt_quarter()
                        scores_exp(nt, h, g, e_t)
                        if first:
                            # prefetch the next KT quarter; stream V in the
                            # slack of the first score groups
                            emit_kt_quarter()
                            if g == 0:
                                backlog.append((emit_ind, 0, 0))
                                # spread V emission: pair mc//2 lands just
                                # before the agg group that first reads it
                                for mc in range(m_ch):
                                    backlog.append(
                                        (lambda mc=mc: emit_v(mc), 128,
                                         mc // 2 + 1))
                        backlog.append(
                            (lambda nt=nt, h=h, g=g, e_t=e_t, A_h=A_h:
                             agg_group(nt, h, g, e_t, A_h), 520,
                             slot[0] + 2))
                        # QT for the next tile must be emitted before that
                        # tile's first score group reads qt_sb (in-order PE)
                        if h == 1 and g == 1 and nt + 1 < n_tiles:
                            emit_qt(nt + 1)
                        # skip pumping right before the next head's first
                        # score group so the score stream never pauses on
                        # the in-order PE; catch up with a double pump after
                        slot[0] += 1
                        if g == n_groups - 1:
                            budget = 0
                        elif g == 0:
                            budget = 2400
                        else:
                            budget = 1200
                        pump(budget if not first else 700)
                    if h == 0:
                        # norm h0 once its last agg group has drained
                        aggT_t = state[("fin", nt)][0]
                        A_h0 = state.pop((nt, 0))
                        backlog.append(
                            (lambda nt=nt, A_h0=A_h0, aggT_t=aggT_t:
                             norm(nt, 0, A_h0, aggT_t), 0, slot[0] + 2))
                    first = False
                if nt < n_tiles - 1:
                    queue_finalize(nt, slot[0])

            # defensive drain (the last-head path drains inline)
            while backlog:
                fn, _, _ = backlog.pop(0)
                fn()

    nc.compile()
    return nc


def shard_inputs(x, source, source_mask, Wq, Wk, Wv, Wm):
    """Build the 8 per-core input dicts. Returns (in_maps, m_pad)."""
    B = x.shape[0]
    valid = [np.flatnonzero(source_mask[b]) for b in range(B)]
    m_pad = max(P, int(math.ceil(max(len(v) for v in valid) / P)) * P)
    m_ch = m_pad // P

    def pack_k_major(w_part):
        # w_part [128 out-dims, 512 in-dims] -> [p, co, d] with k=co*128+p
        return np.ascontiguousarray(
            w_part.T.reshape(4, P, P).transpose(1, 0, 2)).astype(NP_BF16)

    per_batch = []
    for b in range(B):
        v = valid[b]
        xTb = np.ascontiguousarray(
            x[b].T.reshape(4, P, N_SEQ).transpose(1, 0, 2)).astype(NP_BF16)
        sc = np.zeros((m_pad, DIM), np.float32)
        sc[: len(v)] = source[b][v]
        sTb = np.ascontiguousarray(
            sc.T.reshape(4, P, m_pad).transpose(1, 0, 2)).astype(NP_BF16)
        indb = np.zeros((m_pad,), np.float32)
        indb[: len(v)] = 1.0
        ind_pm = np.ascontiguousarray(indb.reshape(m_ch, P).T)  # [p, mc]
        per_batch.append((xTb, sTb, ind_pm))

    in_maps = []
    for core in range(8):
        b, hp = divmod(core, 4)
        rows = slice(hp * P, (hp + 1) * P)
        xTb, sTb, ind_pm = per_batch[b]
        wv_p = pack_k_major(np.asarray(Wv)[rows])          # [p, co, d]
        wm_p = np.asarray(Wm)[:, rows].T.astype(np.float32)  # [p=128, 512]
        wvmi = np.concatenate(
            [wv_p.reshape(P, DIM), wm_p, ind_pm], axis=1).astype(NP_BF16)
        in_maps.append({
            "xT": xTb,
            "sT": sTb,
            "wq": pack_k_major(np.asarray(Wq)[rows]),
            "wk": pack_k_major(np.asarray(Wk)[rows]),
            "wvmi": np.ascontiguousarray(wvmi),
        })
    return in_maps, m_pad


_CACHE = {}


def _get_nc(m_pad):
    if m_pad not in _CACHE:
        _CACHE[m_pad] = build_bass(m_pad)
    return _CACHE[m_pad]


def kernel(x, source, source_mask, Wq, bq, Wk, bk, Wv, bv, Wm, bm,
           _trace=False, **run_kwargs):
    x = np.asarray(x)
    source = np.asarray(source)
    source_mask = np.asarray(source_mask)
    Wq, Wk, Wv, Wm = (np.asarray(a) for a in (Wq, Wk, Wv, Wm))
    bq, bk, bv, bm = (np.asarray(a) for a in (bq, bk, bv, bm))

    # Biases in this problem are structurally zero; the device program folds
    # none, so guard against surprises.
    assert not (np.any(bq) or np.any(bk) or np.any(bv)), \
        "nonzero q/k/v biases not supported by this kernel build"

    in_maps, m_pad = shard_inputs(x, source, source_mask, Wq, Wk, Wv, Wm)
    nc = _get_nc(m_pad)
    res = run_bass_kernel_spmd(nc, in_maps, core_ids=list(range(8)),
                               trace=_trace, **run_kwargs)

    B, N, D = x.shape
    outv = np.zeros((B, N, D), np.float32)
    for core in range(8):
        b = core // 4
        outv[b] += np.asarray(res.results[core]["out"], dtype=np.float32)
    outv += bm.astype(np.float32)
    if _trace:
        kernel._last_results = res
    return outv
